# revision 7
# baseline (speedup 1.0000x reference)
"""Bass/Trainium2 kernel for nn_ConcatAttention (additive attention).

Reference computation (per full problem):
    out_d = input_d @ W_d            # [B, Ld, H]
    out_e = input_e @ W_e            # [B, Le, H]
    t     = tanh(out_d[:,None] + out_e[:,:,None] + b)   # [B, Le, Ld, H]
    energy= t @ v                    # [B, Le, Ld, L]
    out   = transpose(energy, (0,3,2,1))                # [B, L, Ld, Le]

Shapes: B=4, Ld=Le=256, D=512, H=256, L=8.

Sharding: 8 cores = (batch b in 0..3) x (d-half in 0..1). Each core handles
d_local in [0,128), all e, producing out[b, :, d_range, :] = [8, 128, 256].

Per-core pipeline (d-major so the energy PSUM free dim = e = HBM-contiguous).
ScalarE (tanh, the irreducible ~55us of work) is the pacing engine; the
broadcast-add feeding it is split between DVE (tensor_scalar, 2x mode) and
the otherwise-idle TensorE (identity-matmul pairs with stride-0 broadcast rhs
accumulating eb + dT_col directly in PSUM, which ACT reads directly):
  d 0..7  of each 16-d superblock: DVE  adds -> S in SBUF (fp32)
  d 8..15 of each 16-d superblock: PE   adds -> S in PSUM (bf16 inputs)
Then ACT tanh -> bf16 tiles, PE energy matmul (v.T @ tanh) col-group packed
4 d-pairs per PSUM bank, DVE bank evacuation, 4D-AP batched DMA out.
"""

import numpy as np

B, LD, LE = 4, 256, 256
D, H, L = 512, 256, 8
LD_LOC = 128          # d per core
KC = 4                # 512 / 128 k-chunks
HC = 2                # 256 / 128 h-chunks
SBLK = 16             # d per superblock
N_SBLK = LD_LOC // SBLK
D_DVE = 8             # d's per superblock via DVE tensor_scalar path

_CACHE = {}


def _build_nc():
    import concourse.bacc as bacc
    import concourse.mybir as mybir
    import concourse.tile as tile
    from concourse import masks

    fp32 = mybir.dt.float32
    bf16 = mybir.dt.bfloat16
    Tanh = mybir.ActivationFunctionType.Tanh

    nc = bacc.Bacc("TRN2", target_bir_lowering=False, debug=False)

    x_d = nc.dram_tensor("input_d", [LD_LOC, D], fp32, kind="ExternalInput").ap()
    x_e = nc.dram_tensor("input_e", [LE, D], fp32, kind="ExternalInput").ap()
    w_d = nc.dram_tensor("w_d", [D, H], fp32, kind="ExternalInput").ap()
    w_e = nc.dram_tensor("w_e", [D, H], fp32, kind="ExternalInput").ap()
    v_in = nc.dram_tensor("v", [H, L], fp32, kind="ExternalInput").ap()
    b_in = nc.dram_tensor("bias", [H], fp32, kind="ExternalInput").ap()
    out = nc.dram_tensor("out", [L, LD_LOC, LE], fp32, kind="ExternalOutput").ap()
    out_v = out.rearrange("l (blk d) e -> l blk d e", d=8)

    with tile.TileContext(nc) as tc:
        with (
            tc.tile_pool(name="const", bufs=1) as const_pool,
            tc.tile_pool(name="win", bufs=1) as win_pool,
            tc.tile_pool(name="proj", bufs=1) as proj_pool,
            tc.tile_pool(name="s", bufs=3) as s_pool,
            tc.tile_pool(name="tanh", bufs=3) as tanh_pool,
            tc.tile_pool(name="res", bufs=3) as res_pool,
        ):
            # warm the ACT tanh table set immediately (overlaps input DMA)
            warm = const_pool.tile([128, 16], fp32)
            nc.vector.memset(warm[:], 0.0)
            nc.scalar.activation(warm[:], warm[:], Tanh)

            ident16 = const_pool.tile([128, 128], bf16)
            masks.make_identity(nc, ident16[:])
            ident32 = const_pool.tile([128, 128], fp32)
            masks.make_identity(nc, ident32[:])

            bias_sb = const_pool.tile([128, HC], fp32)
            nc.sync.dma_start(bias_sb[:], b_in.rearrange("(c p) -> p c", p=128))
            v_sb = const_pool.tile([128, HC * L], fp32)
            for c in range(HC):
                nc.sync.dma_start(
                    v_sb[:, c * L : (c + 1) * L], v_in[c * 128 : (c + 1) * 128, :]
                )
            v_mm = const_pool.tile([128, HC * 32], bf16)
            nc.vector.memset(v_mm[:], 0.0)
            for c in range(HC):
                nc.vector.tensor_copy(
                    v_mm[:, c * 32 : c * 32 + L], v_sb[:, c * L : (c + 1) * L]
                )

            x_d_sb = win_pool.tile([128, D], fp32)
            nc.sync.dma_start(x_d_sb[:], x_d[:])
            x_e_sb = win_pool.tile([128, 2 * D], fp32)
            for eh in range(2):
                nc.sync.dma_start(
                    x_e_sb[:, eh * D : (eh + 1) * D], x_e[eh * 128 : (eh + 1) * 128, :]
                )
            w_d_sb = win_pool.tile([128, KC * H], fp32)
            w_e_sb = win_pool.tile([128, KC * H], fp32)
            for k in range(KC):
                nc.sync.dma_start(
                    w_e_sb[:, k * H : (k + 1) * H], w_e[k * 128 : (k + 1) * 128, :]
                )
                nc.sync.dma_start(
                    w_d_sb[:, k * H : (k + 1) * H], w_d[k * 128 : (k + 1) * 128, :]
                )

            pro_ctx = tc.tile_pool(name="ps_pro", bufs=3, space="PSUM")
            pspro_pool = pro_ctx.__enter__()

            # Transposes: x_dT[k][i, d], x_eT[k][i, e]
            x_dT = win_pool.tile([128, KC * 128], fp32)
            x_eT = win_pool.tile([128, KC * LE], fp32)
            for eh in range(2):
                for k in range(KC):
                    pt = pspro_pool.tile([128, 256], fp32, tag="pro", name="pt")[:, :128]
                    nc.tensor.transpose(
                        pt[:], x_e_sb[:, eh * D + k * 128 : eh * D + (k + 1) * 128],
                        ident32[:],
                    )
                    nc.vector.tensor_copy(
                        x_eT[:, k * LE + eh * 128 : k * LE + (eh + 1) * 128], pt[:]
                    )
            for k in range(KC):
                pt = pspro_pool.tile([128, 256], fp32, tag="pro", name="pt")[:, :128]
                nc.tensor.transpose(
                    pt[:], x_d_sb[:, k * 128 : (k + 1) * 128], ident32[:]
                )
                nc.vector.tensor_copy(x_dT[:, k * 128 : (k + 1) * 128], pt[:])

            # Projections: out_ebT[c][h_l, e] (+bias) and out_dT[c][h_l, d]
            out_dT = proj_pool.tile([128, HC * 128], fp32)
            out_ebT = proj_pool.tile([128, HC * LE], fp32)
            eb16 = proj_pool.tile([128, HC * LE], bf16)
            dT16 = proj_pool.tile([128, HC * 128], bf16)
            for c in range(HC):
                pe_ps = pspro_pool.tile([128, 256], fp32, tag="pro", name="pe_ps")
                for k in range(KC):
                    nc.tensor.matmul(
                        pe_ps[:],
                        w_e_sb[:, k * H + c * 128 : k * H + (c + 1) * 128],
                        x_eT[:, k * LE : (k + 1) * LE],
                        start=(k == 0),
                        stop=(k == KC - 1),
                    )
                nc.vector.tensor_scalar_add(
                    out_ebT[:, c * LE : (c + 1) * LE], pe_ps[:], bias_sb[:, c : c + 1]
                )
                nc.vector.tensor_copy(
                    eb16[:, c * LE : (c + 1) * LE], out_ebT[:, c * LE : (c + 1) * LE]
                )
            for c in range(HC):
                pd_ps = pspro_pool.tile([128, 256], fp32, tag="pro", name="pd_ps")[:, :128]
                for k in range(KC):
                    nc.tensor.matmul(
                        pd_ps[:],
                        w_d_sb[:, k * H + c * 128 : k * H + (c + 1) * 128],
                        x_dT[:, k * 128 : (k + 1) * 128],
                        start=(k == 0),
                        stop=(k == KC - 1),
                    )
                nc.vector.tensor_copy(out_dT[:, c * 128 : (c + 1) * 128], pd_ps[:])
                nc.vector.tensor_copy(dT16[:, c * 128 : (c + 1) * 128], pd_ps[:])

            pro_ctx.__exit__(None, None, None)
            psS_ctx = tc.tile_pool(name="ps_s", bufs=1, space="PSUM")
            psS_pool = psS_ctx.__enter__()
            pse_ctx = tc.tile_pool(name="ps_e", bufs=3, space="PSUM")
            pse_pool = pse_ctx.__enter__()

            n_pe = SBLK - D_DVE  # d's per superblock via PE path
            for sblk in range(N_SBLK):
                d0 = sblk * SBLK
                t_tiles = []
                for c in range(HC):
                    ebT = out_ebT[:, c * LE : (c + 1) * LE]
                    eb16c = eb16[:, c * LE : (c + 1) * LE]
                    t_t = tanh_pool.tile([128, SBLK * LE], bf16)

                    # DVE path: d0 .. d0+D_DVE-1
                    s_t = s_pool.tile([128, D_DVE * LE], fp32)
                    for i in range(D_DVE):
                        nc.vector.tensor_scalar_add(
                            s_t[:, i * LE : (i + 1) * LE],
                            ebT,
                            out_dT[:, c * 128 + d0 + i : c * 128 + d0 + i + 1],
                        )
                    nc.scalar.activation(t_t[:, : D_DVE * LE], s_t[:], Tanh)

                    # PE path: d0+D_DVE .. d0+SBLK-1 (pairs), S in PSUM
                    s_ps = psS_pool.tile([128, n_pe * LE], fp32, name="s_ps")
                    eb_b = eb16c.rearrange("p (d e) -> p d e", d=1).broadcast_to(
                        [128, 2, LE]
                    )
                    for p in range(n_pe // 2):
                        dd = c * 128 + d0 + D_DVE + 2 * p
                        dt_b = dT16[:, dd : dd + 2].rearrange(
                            "p (d e) -> p d e", e=1
                        ).broadcast_to([128, 2, LE])
                        nc.tensor.matmul(
                            s_ps[:, p * 512 : (p + 1) * 512],
                            ident16[:],
                            eb_b,
                            start=True,
                            stop=False,
                        )
                        nc.tensor.matmul(
                            s_ps[:, p * 512 : (p + 1) * 512],
                            ident16[:],
                            dt_b,
                            start=False,
                            stop=True,
                        )
                    nc.scalar.activation(t_t[:, D_DVE * LE :], s_ps[:], Tanh)
                    t_tiles.append(t_t)

                res = res_pool.tile([128, 1024], fp32)
                for bank in range(2):
                    ps = pse_pool.tile([128, 512], fp32)
                    for j in range(4):
                        pair = bank * 4 + j
                        for c in range(HC):
                            nc.tensor.matmul(
                                ps[32 * j : 32 * j + 32, :],
                                v_mm[:, c * 32 : (c + 1) * 32],
                                t_tiles[c][:, pair * 512 : (pair + 1) * 512],
                                start=(c == 0),
                                stop=(c == HC - 1),
                                tile_position=(0, 32 * j),
                            )
                    nc.vector.tensor_copy(
                        res[:, bank * 512 : (bank + 1) * 512], ps[:]
                    )
                for j in range(4):
                    nc.sync.dma_start(
                        out_v[:, 2 * sblk : 2 * sblk + 2, 2 * j : 2 * j + 2, :],
                        res[32 * j : 32 * j + 8, :].rearrange(
                            "p (blk d e) -> p blk d e", blk=2, e=LE
                        ),
                    )
            pse_ctx.__exit__(None, None, None)
            psS_ctx.__exit__(None, None, None)
    nc.compile()
    return nc


def _get_nc():
    if "nc" not in _CACHE:
        _CACHE["nc"] = _build_nc()
    return _CACHE["nc"]


def make_in_maps(input_d, input_e, W_d, W_e, b, v):
    input_d = np.asarray(input_d, np.float32)
    input_e = np.asarray(input_e, np.float32)
    W_d = np.asarray(W_d, np.float32)
    W_e = np.asarray(W_e, np.float32)
    b = np.asarray(b, np.float32)
    v = np.asarray(v, np.float32)
    in_maps = []
    for c in range(8):
        bi, dh = c // 2, c % 2
        in_maps.append(
            {
                "input_d": np.ascontiguousarray(
                    input_d[bi, dh * LD_LOC : (dh + 1) * LD_LOC, :]
                ),
                "input_e": np.ascontiguousarray(input_e[bi]),
                "w_d": W_d,
                "w_e": W_e,
                "v": v,
                "bias": b,
            }
        )
    return in_maps


def assemble(results):
    out = np.empty((B, L, LD, LE), np.float32)
    for c in range(8):
        bi, dh = c // 2, c % 2
        out[bi, :, dh * LD_LOC : (dh + 1) * LD_LOC, :] = results[c]["out"]
    return out


def kernel(input_d, input_e, W_d, W_e, b, v):
    from concourse.bass_utils import run_bass_kernel_spmd

    nc = _get_nc()
    in_maps = make_in_maps(input_d, input_e, W_d, W_e, b, v)
    res = run_bass_kernel_spmd(nc, in_maps, core_ids=list(range(8)))
    return assemble(res.results)


# revision 8
# speedup vs baseline: 1.3061x; 1.3061x over previous
"""Bass/Trainium2 kernel for nn_ConcatAttention (additive attention).

Reference computation (per full problem):
    out_d = input_d @ W_d            # [B, Ld, H]
    out_e = input_e @ W_e            # [B, Le, H]
    t     = tanh(out_d[:,None] + out_e[:,:,None] + b)   # [B, Le, Ld, H]
    energy= t @ v                    # [B, Le, Ld, L]
    out   = transpose(energy, (0,3,2,1))                # [B, L, Ld, Le]

Shapes: B=4, Ld=Le=256, D=512, H=256, L=8.

Sharding: 8 cores = (batch b in 0..3) x (d-half in 0..1). Each core handles
d_local in [0,128), all e, producing out[b, :, d_range, :] = [8, 128, 256].

Per-core pipeline (d-major so the energy PSUM free dim = e = HBM-contiguous).
ScalarE (tanh, the irreducible ~55us of work) is the pacing engine; the
broadcast-add feeding it is split between DVE (tensor_scalar, 2x mode) and
the otherwise-idle TensorE (identity-matmul pairs with stride-0 broadcast rhs
accumulating eb + dT_col directly in PSUM, which ACT reads directly):
  d 0..7  of each 16-d superblock: DVE  adds -> S in SBUF (fp32)
  d 8..15 of each 16-d superblock: PE   adds -> S in PSUM (bf16 inputs)
Then ACT tanh -> bf16 tiles, PE energy matmul (v.T @ tanh) col-group packed
4 d-pairs per PSUM bank, DVE bank evacuation, 4D-AP batched DMA out.
"""

import numpy as np

B, LD, LE = 4, 256, 256
D, H, L = 512, 256, 8
LD_LOC = 128          # d per core
KC = 4                # 512 / 128 k-chunks
HC = 2                # 256 / 128 h-chunks
SBLK = 16             # d per superblock
N_SBLK = LD_LOC // SBLK
D_DVE = 8             # d's per superblock via DVE tensor_scalar path

_CACHE = {}


def _build_nc():
    import concourse.bacc as bacc
    import concourse.mybir as mybir
    import concourse.tile as tile
    from concourse import masks

    fp32 = mybir.dt.float32
    bf16 = mybir.dt.bfloat16
    Tanh = mybir.ActivationFunctionType.Tanh

    nc = bacc.Bacc("TRN2", target_bir_lowering=False, debug=False)

    x_d = nc.dram_tensor("input_d", [LD_LOC, D], fp32, kind="ExternalInput").ap()
    x_e = nc.dram_tensor("input_e", [LE, D], fp32, kind="ExternalInput").ap()
    w_d = nc.dram_tensor("w_d", [D, H], fp32, kind="ExternalInput").ap()
    w_e = nc.dram_tensor("w_e", [D, H], fp32, kind="ExternalInput").ap()
    v_in = nc.dram_tensor("v", [H, L], fp32, kind="ExternalInput").ap()
    b_in = nc.dram_tensor("bias", [H], fp32, kind="ExternalInput").ap()
    out = nc.dram_tensor("out", [L, LD_LOC, LE], fp32, kind="ExternalOutput").ap()
    out_v = out.rearrange("l (blk d) e -> l blk d e", d=8)

    with tile.TileContext(nc) as tc:
        with (
            tc.tile_pool(name="const", bufs=1) as const_pool,
            tc.tile_pool(name="win", bufs=1) as win_pool,
            tc.tile_pool(name="proj", bufs=1) as proj_pool,
            tc.tile_pool(name="s", bufs=3) as s_pool,
            tc.tile_pool(name="tanh", bufs=5) as tanh_pool,
            tc.tile_pool(name="res", bufs=3) as res_pool,
        ):
            # warm the ACT tanh table set immediately (overlaps input DMA)
            warm = const_pool.tile([128, 16], fp32)
            nc.vector.memset(warm[:], 0.0)
            nc.scalar.activation(warm[:], warm[:], Tanh)

            ident16 = const_pool.tile([128, 128], bf16)
            masks.make_identity(nc, ident16[:])
            ident32 = const_pool.tile([128, 128], fp32)
            masks.make_identity(nc, ident32[:])

            bias_sb = const_pool.tile([128, HC], fp32)
            nc.sync.dma_start(bias_sb[:], b_in.rearrange("(c p) -> p c", p=128))
            v_sb = const_pool.tile([128, HC * L], fp32)
            for c in range(HC):
                nc.sync.dma_start(
                    v_sb[:, c * L : (c + 1) * L], v_in[c * 128 : (c + 1) * 128, :]
                )
            v_mm = const_pool.tile([128, HC * 32], bf16)
            nc.vector.memset(v_mm[:], 0.0)
            for c in range(HC):
                nc.vector.tensor_copy(
                    v_mm[:, c * 32 : c * 32 + L], v_sb[:, c * L : (c + 1) * L]
                )

            x_e_sb = win_pool.tile([128, 2 * D], fp32)
            for eh in range(2):
                nc.sync.dma_start(
                    x_e_sb[:, eh * D : (eh + 1) * D], x_e[eh * 128 : (eh + 1) * 128, :]
                )
            x_d_sb = win_pool.tile([128, D], fp32)
            nc.sync.dma_start(x_d_sb[:], x_d[:])
            w_d_sb = win_pool.tile([128, KC * H], fp32)
            w_e_sb = win_pool.tile([128, KC * H], fp32)
            for k in range(KC):
                nc.sync.dma_start(
                    w_e_sb[:, k * H : (k + 1) * H], w_e[k * 128 : (k + 1) * 128, :]
                )
                nc.sync.dma_start(
                    w_d_sb[:, k * H : (k + 1) * H], w_d[k * 128 : (k + 1) * 128, :]
                )

            pro_ctx = tc.tile_pool(name="ps_pro", bufs=3, space="PSUM")
            pspro_pool = pro_ctx.__enter__()

            # Transposes: x_dT[k][i, d], x_eT[k][i, e]
            x_dT = win_pool.tile([128, KC * 128], fp32)
            x_eT = win_pool.tile([128, KC * LE], fp32)
            for eh in range(2):
                for k in range(KC):
                    pt = pspro_pool.tile([128, 256], fp32, tag="pro", name="pt")[:, :128]
                    nc.tensor.transpose(
                        pt[:], x_e_sb[:, eh * D + k * 128 : eh * D + (k + 1) * 128],
                        ident32[:],
                    )
                    nc.vector.tensor_copy(
                        x_eT[:, k * LE + eh * 128 : k * LE + (eh + 1) * 128], pt[:]
                    )
            for k in range(KC):
                pt = pspro_pool.tile([128, 256], fp32, tag="pro", name="pt")[:, :128]
                nc.tensor.transpose(
                    pt[:], x_d_sb[:, k * 128 : (k + 1) * 128], ident32[:]
                )
                nc.vector.tensor_copy(x_dT[:, k * 128 : (k + 1) * 128], pt[:])

            # Projections: out_ebT[c][h_l, e] (+bias) and out_dT[c][h_l, d]
            out_dT = proj_pool.tile([128, HC * 128], fp32)
            out_ebT = proj_pool.tile([128, HC * LE], fp32)
            eb16 = proj_pool.tile([128, HC * LE], bf16)
            dT16 = proj_pool.tile([128, HC * 128], bf16)
            for c in range(HC):
                pe_ps = pspro_pool.tile([128, 256], fp32, tag="pro", name="pe_ps")
                for k in range(KC):
                    nc.tensor.matmul(
                        pe_ps[:],
                        w_e_sb[:, k * H + c * 128 : k * H + (c + 1) * 128],
                        x_eT[:, k * LE : (k + 1) * LE],
                        start=(k == 0),
                        stop=(k == KC - 1),
                    )
                nc.vector.tensor_scalar_add(
                    out_ebT[:, c * LE : (c + 1) * LE], pe_ps[:], bias_sb[:, c : c + 1]
                )
                nc.vector.tensor_copy(
                    eb16[:, c * LE : (c + 1) * LE], out_ebT[:, c * LE : (c + 1) * LE]
                )
            for c in range(HC):
                pd_ps = pspro_pool.tile([128, 256], fp32, tag="pro", name="pd_ps")[:, :128]
                for k in range(KC):
                    nc.tensor.matmul(
                        pd_ps[:],
                        w_d_sb[:, k * H + c * 128 : k * H + (c + 1) * 128],
                        x_dT[:, k * 128 : (k + 1) * 128],
                        start=(k == 0),
                        stop=(k == KC - 1),
                    )
                nc.vector.tensor_copy(out_dT[:, c * 128 : (c + 1) * 128], pd_ps[:])
                nc.vector.tensor_copy(dT16[:, c * 128 : (c + 1) * 128], pd_ps[:])

            pro_ctx.__exit__(None, None, None)
            psS_ctx = tc.tile_pool(name="ps_s", bufs=3, space="PSUM")
            psS_pool = psS_ctx.__enter__()
            pse_ctx = tc.tile_pool(name="ps_e", bufs=2, space="PSUM")
            pse_pool = pse_ctx.__enter__()

            n_pe = SBLK - D_DVE  # d's per superblock via PE path
            tanh_tiles = {}

            def emit_front(sblk):
                d0 = sblk * SBLK
                t_pair = []
                for c in range(HC):
                    ebT = out_ebT[:, c * LE : (c + 1) * LE]
                    eb16c = eb16[:, c * LE : (c + 1) * LE]
                    t_t = tanh_pool.tile([128, SBLK * LE], bf16, name="t_t")

                    # PE path first: d0+D_DVE .. d0+SBLK-1 in pairs, S in PSUM
                    eb_b = eb16c.rearrange("p (d e) -> p d e", d=1).broadcast_to(
                        [128, 2, LE]
                    )
                    for half in range(n_pe // 4):
                        s_ps = psS_pool.tile([128, 1024], fp32, name="s_ps")
                        for q in range(2):
                            p = half * 2 + q
                            dd = c * 128 + d0 + D_DVE + 2 * p
                            dt_b = dT16[:, dd : dd + 2].rearrange(
                                "p (d e) -> p d e", e=1
                            ).broadcast_to([128, 2, LE])
                            nc.tensor.matmul(
                                s_ps[:, q * 512 : (q + 1) * 512],
                                ident16[:],
                                eb_b,
                                start=True,
                                stop=False,
                            )
                            nc.tensor.matmul(
                                s_ps[:, q * 512 : (q + 1) * 512],
                                ident16[:],
                                dt_b,
                                start=False,
                                stop=True,
                            )
                        nc.scalar.activation(
                            t_t[:, (D_DVE + half * 4) * LE : (D_DVE + half * 4 + 4) * LE],
                            s_ps[:],
                            Tanh,
                        )

                    # DVE path: d0 .. d0+D_DVE-1
                    s_t = s_pool.tile([128, D_DVE * LE], fp32, name="s_t")
                    for i in range(D_DVE):
                        nc.vector.tensor_scalar_add(
                            s_t[:, i * LE : (i + 1) * LE],
                            ebT,
                            out_dT[:, c * 128 + d0 + i : c * 128 + d0 + i + 1],
                        )
                    nc.scalar.activation(t_t[:, : D_DVE * LE], s_t[:], Tanh)
                    t_pair.append(t_t)
                tanh_tiles[sblk] = t_pair

            def emit_back(sblk):
                t_pair = tanh_tiles.pop(sblk)
                res = res_pool.tile([128, 1024], fp32, name="res")
                for bank in range(2):
                    ps = pse_pool.tile([128, 512], fp32, name="ps")
                    for j in range(4):
                        pair = bank * 4 + j
                        for c in range(HC):
                            nc.tensor.matmul(
                                ps[32 * j : 32 * j + 32, :],
                                v_mm[:, c * 32 : (c + 1) * 32],
                                t_pair[c][:, pair * 512 : (pair + 1) * 512],
                                start=(c == 0),
                                stop=(c == HC - 1),
                                tile_position=(0, 32 * j),
                            )
                    nc.vector.tensor_copy(
                        res[:, bank * 512 : (bank + 1) * 512], ps[:]
                    )
                for j in range(4):
                    nc.sync.dma_start(
                        out_v[:, 2 * sblk : 2 * sblk + 2, 2 * j : 2 * j + 2, :],
                        res[32 * j : 32 * j + 8, :].rearrange(
                            "p (blk d e) -> p blk d e", blk=2, e=LE
                        ),
                    )

            for sblk in range(N_SBLK + 1):
                if sblk < N_SBLK:
                    emit_front(sblk)
                if sblk > 0:
                    emit_back(sblk - 1)
            pse_ctx.__exit__(None, None, None)
            psS_ctx.__exit__(None, None, None)
    nc.compile()
    return nc


def _get_nc():
    if "nc" not in _CACHE:
        _CACHE["nc"] = _build_nc()
    return _CACHE["nc"]


def make_in_maps(input_d, input_e, W_d, W_e, b, v):
    input_d = np.asarray(input_d, np.float32)
    input_e = np.asarray(input_e, np.float32)
    W_d = np.asarray(W_d, np.float32)
    W_e = np.asarray(W_e, np.float32)
    b = np.asarray(b, np.float32)
    v = np.asarray(v, np.float32)
    in_maps = []
    for c in range(8):
        bi, dh = c // 2, c % 2
        in_maps.append(
            {
                "input_d": np.ascontiguousarray(
                    input_d[bi, dh * LD_LOC : (dh + 1) * LD_LOC, :]
                ),
                "input_e": np.ascontiguousarray(input_e[bi]),
                "w_d": W_d,
                "w_e": W_e,
                "v": v,
                "bias": b,
            }
        )
    return in_maps


def assemble(results):
    out = np.empty((B, L, LD, LE), np.float32)
    for c in range(8):
        bi, dh = c // 2, c % 2
        out[bi, :, dh * LD_LOC : (dh + 1) * LD_LOC, :] = results[c]["out"]
    return out


def kernel(input_d, input_e, W_d, W_e, b, v):
    from concourse.bass_utils import run_bass_kernel_spmd

    nc = _get_nc()
    in_maps = make_in_maps(input_d, input_e, W_d, W_e, b, v)
    res = run_bass_kernel_spmd(nc, in_maps, core_ids=list(range(8)))
    return assemble(res.results)
